# revision 10
# baseline (speedup 1.0000x reference)
"""Trainium2 Bass kernel for nn_Attn_Module (B=8, C=512, L=2048, CP=64).

Data-parallel over batch: each of the 8 NeuronCores computes one batch element's
full attention. No collectives.

Per-core math (b = batch element):
  v  = value_w @ x[b]                [64, 2048]
  q' = (query_w/32) @ v              [64, 2048]   (1/32 logit scale folded into weights)
  k  = key_w @ v                     [64, 2048]
  E' = q'^T k = E/32                 (computed per l-tile for row stats only)
  row bound b_l = -max_m E'[l, m]    (DVE tiles) or -(ln sum exp E' - 2) (ACT tiles)
  E^T_biased[j, l] = k^T q' + ones*b_row   (bias rides the matmul as a 65th K-row)
  P^T = exp(32 * E^T_biased)         bf16, directly in AV-ready [j, l] layout
  O65 = vT65^T @ P^T accumulated over j-tiles; vT65 = [gamma*v^T | ones-col]
        rows 0-63 = gamma*out_unnorm, row 64 = S2 (softmax denominator)
  out[0:64]  = O65[0:64] / S2 ;  out[64:128] = v
"""
import sys
import types

sys.path.insert(0, '/opt/trn_rl_repo')
sys.path.insert(0, '/root/.axon_site')

import numpy as np


def _install_ntff_hook():
    try:
        import antenv
    except ImportError:
        return
    if 'antenv.axon_hooks' in sys.modules:
        return
    mod = types.ModuleType('antenv.axon_hooks')
    mod._hook = None
    mod.set_axon_ntff_profile_hook = lambda h: setattr(mod, '_hook', h)
    mod.get_axon_ntff_profile_hook = lambda: mod._hook
    sys.modules['antenv.axon_hooks'] = mod
    antenv.axon_hooks = mod
    try:
        from trn_agent_boot.trn_boot import _ntff_profile_via_ctypes
        mod.set_axon_ntff_profile_hook(_ntff_profile_via_ctypes('/opt/axon/libaxon_pjrt.so'))
    except Exception:
        pass


_install_ntff_hook()

import concourse.bacc as bacc
import concourse.mybir as mybir
from concourse.bass_utils import run_bass_kernel_spmd
from concourse.tile import TileContext

F32 = mybir.dt.float32
F32R = mybir.dt.float32r
BF16 = mybir.dt.bfloat16

B, C, L, CP = 8, 512, 2048, 64
NLT = L // 128
NJT = L // 128
NLC = L // 512
SCALE = 32.0

# stat style per l-tile PAIR (8 pairs): True = ACT LSE, False = DVE max
PAIR_ON_ACT = [False, True, False, False, True, False, False, True]


def f32r_round(a):
    """Round fp32 array to the float32r grid (RNE on low 12 mantissa bits, sign-magnitude)."""
    a = np.ascontiguousarray(a, np.float32)
    xi = a.view(np.int32)
    sign = xi & np.int32(-2**31)
    mag = (xi & np.int32(0x7FFFFFFF)).astype(np.int64)
    add = 1 << 11
    mr = mag + add
    ties = (mag & ((1 << 12) - 1)) == add
    mr = np.where(ties & (((mag >> 12) & 1) == 0), mag, mr)
    mr &= ~((1 << 12) - 1)
    return (sign | mr.astype(np.int32)).view(np.float32).reshape(a.shape)


def build_nc(gamma: float):
    nc = bacc.Bacc()
    x_p = nc.declare_dram_parameter('x', [C, L], F32R, isOutput=False)
    vwT_p = nc.declare_dram_parameter('vwT', [C, CP], F32R, isOutput=False)
    qwT_p = nc.declare_dram_parameter('qwT', [CP, CP], F32R, isOutput=False)
    kwT_p = nc.declare_dram_parameter('kwT', [CP, CP], F32R, isOutput=False)
    id_p = nc.declare_dram_parameter('ident', [128, 128], F32R, isOutput=False)
    out_p = nc.declare_dram_parameter('out', [128, L], F32, isOutput=True)

    LNC = 2.0 - 24 * float(np.log(2.0))

    with TileContext(nc) as tc:
        with tc.tile_pool(name='sb', bufs=1) as sb, \
             tc.tile_pool(name='pt', bufs=6) as ptp, \
             tc.tile_pool(name='scr', bufs=4) as scr, \
             tc.tile_pool(name='wk', bufs=2, space='PSUM') as wkp, \
             tc.tile_pool(name='oo', bufs=1, space='PSUM') as oo:

            # ---------- loads ----------
            ident = sb.tile([128, 128], F32R, tag='ident')
            nc.sync.dma_start(ident[:], id_p[:])
            xt = [sb.tile([128, L], F32R, tag=f'x{kt}', name=f'x{kt}') for kt in range(4)]
            for lc in range(NLC):
                for kt in range(4):
                    eng = nc.sync if (kt + lc) % 2 == 0 else nc.scalar
                    eng.dma_start(xt[kt][:, lc * 512:(lc + 1) * 512],
                                  x_p[kt * 128:(kt + 1) * 128, lc * 512:(lc + 1) * 512])
            vw = sb.tile([128, 4 * CP], F32R, tag='vw')
            for kt in range(4):
                nc.sync.dma_start(vw[:, kt * CP:(kt + 1) * CP], vwT_p[kt * 128:(kt + 1) * 128, :])
            qkw = sb.tile([64, 2 * CP], F32R, tag='qkw')
            nc.sync.dma_start(qkw[:, 0:CP], qwT_p[:])
            nc.sync.dma_start(qkw[:, CP:2 * CP], kwT_p[:])

            # ---------- v = value_w @ x (chunked, dup DMAs per chunk) ----------
            v_sb = sb.tile([64, L], F32R, tag='v')
            for lc in range(NLC):
                pv = wkp.tile([64, 512], F32, tag='wk', name=f'pv{lc}')
                for kt in range(4):
                    nc.tensor.matmul(pv[:], vw[:, kt * CP:(kt + 1) * CP],
                                     xt[kt][:, lc * 512:(lc + 1) * 512],
                                     start=(kt == 0), stop=(kt == 3))
                nc.vector.tensor_copy(v_sb[:, lc * 512:(lc + 1) * 512], pv[:])

            # ---------- vT65 (v^T * gamma | ones col), bf16 ----------
            vt65 = sb.tile([128, NJT * 65], BF16, tag='vt65')
            for g in range(2):
                pvt = wkp.tile([128, 512], F32R, tag='wk', name=f'pvt{g}')
                for bi in range(8):
                    jt = g * 8 + bi
                    nc.tensor.transpose(pvt[:, bi * 64:(bi + 1) * 64],
                                        v_sb[:, jt * 128:(jt + 1) * 128],
                                        ident[0:64, 0:64])
                dst = vt65[:, g * 8 * 65:].rearrange('p (a b) -> p a b', b=65)[:, 0:8, 0:64]
                nc.vector.tensor_scalar_mul(dst, pvt[:].rearrange('p (a b) -> p a b', b=64), float(gamma))
            ones_col = vt65[:].rearrange('p (a b) -> p a b', b=65)[:, :, 64:65]
            nc.gpsimd.memset(ones_col, 1.0)

            # ---------- q', k; build QQ/KK (packed-stat operands) and Q65/K65 ----------
            QQ = sb.tile([128, L], F32R, tag='QQ')
            KK = sb.tile([128, L], F32R, tag='KK')
            Q65 = sb.tile([128, L], F32R, tag='Q65')   # row 0 = bias row (written later)
            K65 = sb.tile([128, L], F32R, tag='K65')   # row 0 = ones
            nc.gpsimd.memset(K65[0:1, :].bitcast(F32), 1.0)
            for lc in range(NLC):
                sl = slice(lc * 512, (lc + 1) * 512)
                pqk = wkp.tile([128, 1024], F32, tag='wk', name=f'pqk{lc}')
                pq = pqk[0:64, 0:512]
                pk = pqk[0:64, 512:1024]
                nc.tensor.matmul(pq, qkw[:, 0:CP], v_sb[:, sl], start=True, stop=True)
                nc.tensor.matmul(pk, qkw[:, CP:2 * CP], v_sb[:, sl], start=True, stop=True)
                nc.vector.tensor_copy(QQ[0:64, sl], pq)
                nc.scalar.copy(KK[0:64, sl], pk)
                e1 = nc.sync if lc % 2 == 0 else nc.scalar
                e2 = nc.scalar if lc % 2 == 0 else nc.sync
                e1.dma_start(QQ[64:128, sl], QQ[0:64, sl])
                e2.dma_start(KK[64:128, sl], KK[0:64, sl])
                e1.dma_start(Q65[1:65, sl], QQ[0:64, sl])
                e2.dma_start(K65[1:65, sl], KK[0:64, sl])

            stats = sb.tile([128, NLT], F32R, tag='stats')

            # ---------- stats for one l-chunk (2 tile pairs) ----------
            def emit_stats(lc):
                for half in range(2):
                    pair = lc * 2 + half
                    ltA, ltB = 2 * pair, 2 * pair + 1
                    on_act = PAIR_ON_ACT[pair]
                    mx = scr.tile([128, 8], F32, tag='mx', name=f'mx{pair}')
                    for mc in range(NLC):
                        pp = wkp.tile([128, 1024], F32, tag='wk', name=f'pp{pair}_{mc}')
                        nc.tensor.matmul(pp[:, 0:512], QQ[0:64, ltA * 128:(ltA + 1) * 128],
                                         KK[0:64, mc * 512:(mc + 1) * 512], start=True, stop=True)
                        nc.tensor.matmul(pp[:, 512:1024], QQ[64:128, ltB * 128:(ltB + 1) * 128],
                                         KK[64:128, mc * 512:(mc + 1) * 512], start=True, stop=True)
                        if on_act:
                            s = scr.tile([128, 1024], BF16, tag='scrp1', name=f's{pair}_{mc}')
                            nc.scalar.activation(s[:, 0:512], pp[:, 0:512],
                                                 mybir.ActivationFunctionType.Exp,
                                                 bias=0.0, scale=1.0, accum_out=mx[:, 2 * mc:2 * mc + 1])
                            nc.scalar.activation(s[:, 512:1024], pp[:, 512:1024],
                                                 mybir.ActivationFunctionType.Exp,
                                                 bias=0.0, scale=1.0, accum_out=mx[:, 2 * mc + 1:2 * mc + 2])
                        else:
                            nc.vector.reduce_max(
                                mx[:, 2 * mc:2 * mc + 2],
                                pp[:].rearrange('p (a b) -> p a b', a=2),
                                axis=mybir.AxisListType.X)
                    for i, lt in ((0, ltA), (1, ltB)):
                        b1 = scr.tile([128, 1], F32, tag='b1', name=f'b1_{lt}')
                        sub = mx[:].rearrange('p (a b) -> p a b', b=2)[:, :, i:i + 1]
                        if on_act:
                            nc.vector.reduce_sum(b1[:], sub, axis=mybir.AxisListType.XY)
                            nc.vector.tensor_scalar_mul(b1[:], b1[:], 2.0 ** -24)
                            nc.scalar.activation(b1[:], b1[:], mybir.ActivationFunctionType.Ln)
                            nc.vector.tensor_scalar(stats[:, lt:lt + 1], b1[:],
                                                    scalar1=-1.0, scalar2=LNC,
                                                    op0=mybir.AluOpType.mult,
                                                    op1=mybir.AluOpType.add)
                        else:
                            nc.vector.reduce_max(b1[:], sub, axis=mybir.AxisListType.XY)
                            nc.vector.tensor_scalar_mul(stats[:, lt:lt + 1], b1[:], -1.0)

            def emit_brow(lc):
                pb = wkp.tile([4, 128], F32R, tag='wk', name=f'pb{lc}')
                nc.tensor.transpose(pb[:], stats[:, lc * 4:(lc + 1) * 4], ident[:])
                bs = scr.tile([4, 128], F32R, tag='bs', name=f'bs{lc}')
                nc.vector.tensor_copy(bs[:], pb[:])
                nc.sync.dma_start(
                    Q65[0:1, lc * 512:(lc + 1) * 512].rearrange('p (a b) -> p a b', b=128),
                    bs[:])

            o65 = [oo.tile([65, 512], F32, tag=f'o{lc}', name=f'o65_{lc}') for lc in range(NLC)]

            emit_stats(0)
            emit_brow(0)
            emit_stats(1)
            def emit_norm(lc):
                r1 = scr.tile([1, 512], F32, tag='r1', name=f'r1_{lc}')
                nc.vector.reciprocal(r1[:], o65[lc][64:65, :])
                r2 = scr.tile([64, 512], F32, tag='r2', name=f'r2_{lc}')
                nc.gpsimd.partition_broadcast(r2[:], r1[:])
                ofin = scr.tile([64, 512], F32, tag='ofin', name=f'of{lc}')
                nc.vector.tensor_tensor(ofin[:], o65[lc][0:64, :], r2[:], op=mybir.AluOpType.mult)
                nc.sync.dma_start(out_p[0:64, lc * 512:(lc + 1) * 512], ofin[:])

            for lc in range(NLC):
                # C(lc): per jt-pair: E^T x2 into one [128,1024] psum, one exp, two AV MMs
                sl = slice(lc * 512, (lc + 1) * 512)
                pts = []
                for jp in range(NJT // 2):
                    j0, j1 = 2 * jp, 2 * jp + 1
                    e = wkp.tile([128, 1024], F32, tag='wk', name=f'e{lc}_{jp}')
                    nc.tensor.matmul(e[:, 0:512], K65[0:65, j0 * 128:(j0 + 1) * 128],
                                     Q65[0:65, sl], start=True, stop=True)
                    nc.tensor.matmul(e[:, 512:1024], K65[0:65, j1 * 128:(j1 + 1) * 128],
                                     Q65[0:65, sl], start=True, stop=True)
                    pt = ptp.tile([128, 1024], BF16, tag='pt', name=f'pt{lc}_{jp}')
                    nc.scalar.activation(pt[:], e[:], mybir.ActivationFunctionType.Exp,
                                         bias=0.0, scale=SCALE)
                    pts.append(pt)
                    # AV for the PREVIOUS pair (keeps PE fed while ACT works on this one)
                    if jp > 0:
                        pprev = pts[jp - 1]
                        for i, j in ((0, 2 * jp - 2), (1, 2 * jp - 1)):
                            nc.tensor.matmul(o65[lc][:], vt65[:, j * 65:(j + 1) * 65],
                                             pprev[:, i * 512:(i + 1) * 512],
                                             start=(j == 0), stop=False)
                plast = pts[-1]
                for i, j in ((0, NJT - 2), (1, NJT - 1)):
                    nc.tensor.matmul(o65[lc][:], vt65[:, j * 65:(j + 1) * 65],
                                     plast[:, i * 512:(i + 1) * 512],
                                     start=False, stop=(j == NJT - 1))
                if lc + 1 < NLC:
                    emit_brow(lc + 1)
                if lc + 2 < NLC:
                    emit_stats(lc + 2)
                if lc >= 1:
                    emit_norm(lc - 1)
            emit_norm(NLC - 1)

            # v output channels: straight DMA (f32r bits are valid f32)
            nc.scalar.dma_start(out_p[64:128, :], v_sb[:].bitcast(F32))

    nc.finalize()
    return nc


_cache = {}


def _get_nc(gamma: float):
    key = float(gamma)
    if key not in _cache:
        _cache[key] = build_nc(key)
    return _cache[key]


def _in_maps(inputs):
    x = np.asarray(inputs['x'], np.float32)
    vwT = f32r_round(np.asarray(inputs['value_w'], np.float32).T)
    qwT = f32r_round(np.asarray(inputs['query_w'], np.float32).T / SCALE)
    kwT = f32r_round(np.asarray(inputs['key_w'], np.float32).T)
    ident = np.eye(128, dtype=np.float32)
    xs = f32r_round(x[..., 0])
    return [
        {'x': np.ascontiguousarray(xs[b]), 'vwT': vwT, 'qwT': qwT, 'kwT': kwT, 'ident': ident}
        for b in range(B)
    ]


def kernel(x, value_w, value_b, query_w, query_b, key_w, key_b, gamma):
    gamma_f = float(np.asarray(gamma).reshape(-1)[0])
    nc = _get_nc(gamma_f)
    maps = _in_maps(dict(x=x, value_w=value_w, query_w=query_w, key_w=key_w))
    res = run_bass_kernel_spmd(nc, maps, core_ids=list(range(B)), trace=False)
    out = np.stack([res.results[b]['out'] for b in range(B)], axis=0)
    return out[..., None].astype(np.float32)


def run_traced(inputs):
    gamma_f = float(np.asarray(inputs['gamma']).reshape(-1)[0])
    nc = _get_nc(gamma_f)
    maps = _in_maps(inputs)
    res = run_bass_kernel_spmd(nc, maps, core_ids=list(range(B)), trace=True)
    out = np.stack([res.results[b]['out'] for b in range(B)], axis=0)
    return out[..., None].astype(np.float32), res.exec_time_ns


# revision 26
# speedup vs baseline: 1.4555x; 1.4555x over previous
"""Trainium2 Bass kernel for nn_Attn_Module (B=8, C=512, L=2048, CP=64).

Data-parallel over batch: each of the 8 NeuronCores computes one batch element's
full attention. No collectives.

Per-core math (b = batch element):
  v  = value_w @ x[b]                [64, 2048]
  q' = (query_w/32) @ v              [64, 2048]   (1/32 logit scale folded into weights)
  k  = key_w @ v                     [64, 2048]
  E' = q'^T k = E/32                 (computed per l-tile for row stats only)
  row bound b_l = -max_m E'[l, m]    (DVE tiles) or -(ln sum exp E' - 2) (ACT tiles)
  E^T_biased[j, l] = k^T q' + ones*b_row   (bias rides the matmul as a 65th K-row)
  P^T = exp(32 * E^T_biased)         bf16, directly in AV-ready [j, l] layout
  O65 = vT65^T @ P^T accumulated over j-tiles; vT65 = [gamma*v^T | ones-col]
        rows 0-63 = gamma*out_unnorm, row 64 = S2 (softmax denominator)
  out[0:64]  = O65[0:64] / S2 ;  out[64:128] = v
"""
import sys
import types

sys.path.insert(0, '/opt/trn_rl_repo')
sys.path.insert(0, '/root/.axon_site')

import numpy as np


def _install_ntff_hook():
    try:
        import antenv
    except ImportError:
        return
    if 'antenv.axon_hooks' in sys.modules:
        return
    mod = types.ModuleType('antenv.axon_hooks')
    mod._hook = None
    mod.set_axon_ntff_profile_hook = lambda h: setattr(mod, '_hook', h)
    mod.get_axon_ntff_profile_hook = lambda: mod._hook
    sys.modules['antenv.axon_hooks'] = mod
    antenv.axon_hooks = mod
    try:
        from trn_agent_boot.trn_boot import _ntff_profile_via_ctypes
        mod.set_axon_ntff_profile_hook(_ntff_profile_via_ctypes('/opt/axon/libaxon_pjrt.so'))
    except Exception:
        pass


_install_ntff_hook()

import concourse.bacc as bacc
import concourse.mybir as mybir
from concourse.bass_utils import run_bass_kernel_spmd
from concourse.tile import TileContext

F32 = mybir.dt.float32
F32R = mybir.dt.float32r
BF16 = mybir.dt.bfloat16

B, C, L, CP = 8, 512, 2048, 64
NLT = L // 128
NJT = L // 128
NLC = L // 512
SCALE = 32.0

# stat style per l-tile PAIR (8 pairs): True = ACT LSE, False = DVE max
PAIR_ON_ACT = [False] * 8


def f32r_round(a):
    """Round fp32 array to the float32r grid (RNE on low 12 mantissa bits, sign-magnitude)."""
    a = np.ascontiguousarray(a, np.float32)
    xi = a.view(np.int32)
    sign = xi & np.int32(-2**31)
    mag = (xi & np.int32(0x7FFFFFFF)).astype(np.int64)
    add = 1 << 11
    mr = mag + add
    ties = (mag & ((1 << 12) - 1)) == add
    mr = np.where(ties & (((mag >> 12) & 1) == 0), mag, mr)
    mr &= ~((1 << 12) - 1)
    return (sign | mr.astype(np.int32)).view(np.float32).reshape(a.shape)


def build_nc(gamma: float):
    nc = bacc.Bacc()
    x_p = nc.declare_dram_parameter('x', [C, L], F32R, isOutput=False)
    vwT_p = nc.declare_dram_parameter('vwT', [C, CP], F32R, isOutput=False)
    qwT_p = nc.declare_dram_parameter('qwT', [CP, CP], F32R, isOutput=False)
    kwT_p = nc.declare_dram_parameter('kwT', [CP, CP], F32R, isOutput=False)
    id_p = nc.declare_dram_parameter('ident', [128, 128], F32R, isOutput=False)
    out_p = nc.declare_dram_parameter('out', [128, L], F32, isOutput=True)

    LNC = 2.0 - 24 * float(np.log(2.0))

    with TileContext(nc) as tc:
        with tc.tile_pool(name='sb', bufs=1) as sb, \
             tc.tile_pool(name='pt', bufs=8) as ptp, \
             tc.tile_pool(name='scr', bufs=4) as scr, \
             tc.tile_pool(name='wk', bufs=2, space='PSUM') as wkp, \
             tc.tile_pool(name='et', bufs=2, space='PSUM') as etp, \
             tc.tile_pool(name='oo', bufs=2, space='PSUM') as oo:

            # ---------- loads ----------
            ident = sb.tile([128, 128], F32R, tag='ident')
            nc.sync.dma_start(ident[:], id_p[:])
            xt = [sb.tile([128, L], F32R, tag=f'x{kt}', name=f'x{kt}') for kt in range(4)]
            for lc in range(NLC):
                for kt in range(4):
                    nc.sync.dma_start(xt[kt][:, lc * 512:(lc + 1) * 512],
                                      x_p[kt * 128:(kt + 1) * 128, lc * 512:(lc + 1) * 512])
            vw = sb.tile([128, 4 * CP], F32R, tag='vw')
            for kt in range(4):
                nc.sync.dma_start(vw[:, kt * CP:(kt + 1) * CP], vwT_p[kt * 128:(kt + 1) * 128, :])
            qkw = sb.tile([64, 2 * CP], F32R, tag='qkw')
            nc.sync.dma_start(qkw[:, 0:CP], qwT_p[:])
            nc.sync.dma_start(qkw[:, CP:2 * CP], kwT_p[:])

            # ---------- PE warmup: keep HAM un-throttled through the DMA ramp ----------
            warm = wkp.tile([128, 128], F32, tag='wk', name='warm')
            for i in range(40):
                nc.tensor.matmul(warm[:], ident[:], ident[:], start=True, stop=True)

            # ---------- v = value_w @ x (chunked, dup DMAs per chunk) ----------
            v_sb = sb.tile([64, L], F32R, tag='v')
            for lc in range(NLC):
                pv = wkp.tile([64, 512], F32, tag='wk', name=f'pv{lc}')
                for kt in range(4):
                    nc.tensor.matmul(pv[:], vw[:, kt * CP:(kt + 1) * CP],
                                     xt[kt][:, lc * 512:(lc + 1) * 512],
                                     start=(kt == 0), stop=(kt == 3))
                nc.scalar.copy(v_sb[:, lc * 512:(lc + 1) * 512], pv[:])

            # ---------- vT65 (v^T * gamma | ones col), bf16 ----------
            vt65 = sb.tile([128, NJT * 65], BF16, tag='vt65')
            for g in range(2):
                pvt = wkp.tile([128, 512], F32R, tag='wk', name=f'pvt{g}')
                for bi in range(8):
                    jt = g * 8 + bi
                    nc.tensor.transpose(pvt[:, bi * 64:(bi + 1) * 64],
                                        v_sb[:, jt * 128:(jt + 1) * 128],
                                        ident[0:64, 0:64])
                dst = vt65[:, g * 8 * 65:].rearrange('p (a b) -> p a b', b=65)[:, 0:8, 0:64]
                nc.scalar.mul(dst, pvt[:].rearrange('p (a b) -> p a b', b=64), float(gamma))
            ones_col = vt65[:].rearrange('p (a b) -> p a b', b=65)[:, :, 64:65]
            nc.gpsimd.memset(ones_col, 1.0)

            # ---------- q', k and Q65/K65 ----------
            q_sb = sb.tile([64, L], F32R, tag='q_sb')
            k_sb = sb.tile([64, L], F32R, tag='k_sb')
            Q65 = sb.tile([128, L], F32R, tag='Q65')   # row 0 = bias row (written later)
            K65 = sb.tile([128, L], F32R, tag='K65')   # row 0 = ones
            nc.gpsimd.memset(K65[0:1, :].bitcast(F32), 1.0)
            for lc in range(NLC):
                sl = slice(lc * 512, (lc + 1) * 512)
                pq = wkp.tile([64, 512], F32, tag='wk', name=f'pq{lc}')
                pk = wkp.tile([64, 512], F32, tag='wk', name=f'pk{lc}')
                nc.tensor.matmul(pq[:], qkw[:, 0:CP], v_sb[:, sl], start=True, stop=True)
                nc.tensor.matmul(pk[:], qkw[:, CP:2 * CP], v_sb[:, sl], start=True, stop=True)
                nc.scalar.copy(q_sb[:, sl], pq[:])
                nc.scalar.copy(k_sb[:, sl], pk[:])
                e1 = nc.sync if lc % 2 == 0 else nc.scalar
                e2 = nc.scalar if lc % 2 == 0 else nc.sync
                e1.dma_start(Q65[1:65, sl], q_sb[:, sl])
                e2.dma_start(K65[1:65, sl], k_sb[:, sl])

            stats = sb.tile([128, NLT], F32R, tag='stats')

            # ---------- stats: one (pair, mc) step; 8 steps per l-chunk ----------
            def stat_steps(lc):
                """Return a list of closures; each emits one stat matmul-pair + reduce."""
                steps = []
                for half in range(2):
                    pair = lc * 2 + half
                    ltA, ltB = 2 * pair, 2 * pair + 1
                    mx = scr.tile([128, 8], F32, tag='mx', name=f'mx{pair}')

                    def mk_mc(pair, ltA, ltB, mx, mc, last):
                        def step():
                            ppA = wkp.tile([128, 512], F32, tag='wk', name=f'ppA{pair}_{mc}')
                            ppB = wkp.tile([128, 512], F32, tag='wk', name=f'ppB{pair}_{mc}')
                            nc.tensor.matmul(ppA[:], q_sb[:, ltA * 128:(ltA + 1) * 128],
                                             k_sb[:, mc * 512:(mc + 1) * 512], start=True, stop=True)
                            nc.tensor.matmul(ppB[:], q_sb[:, ltB * 128:(ltB + 1) * 128],
                                             k_sb[:, mc * 512:(mc + 1) * 512], start=True, stop=True)
                            nc.vector.reduce_max(mx[:, 2 * mc:2 * mc + 1], ppA[:],
                                                 axis=mybir.AxisListType.X)
                            nc.vector.reduce_max(mx[:, 2 * mc + 1:2 * mc + 2], ppB[:],
                                                 axis=mybir.AxisListType.X)
                            if last:
                                for i, lt in ((0, ltA), (1, ltB)):
                                    sub = mx[:].rearrange('p (a b) -> p a b', b=2)[:, :, i:i + 1]
                                    nc.vector.reduce_max(stats[:, lt:lt + 1], sub,
                                                         axis=mybir.AxisListType.XY, negate=True)
                        return step
                    for mc in range(NLC):
                        steps.append(mk_mc(pair, ltA, ltB, mx, mc, mc == NLC - 1))
                return steps

            def emit_brow(lc):
                pb = wkp.tile([4, 128], F32R, tag='wk', name=f'pb{lc}')
                nc.tensor.transpose(pb[:], stats[:, lc * 4:(lc + 1) * 4], ident[:])
                bs = scr.tile([4, 128], F32R, tag='bs', name=f'bs{lc}')
                nc.vector.tensor_copy(bs[:], pb[:])
                nc.sync.dma_start(
                    Q65[0:1, lc * 512:(lc + 1) * 512].rearrange('p (a b) -> p a b', b=128),
                    bs[:])

            o65 = [oo.tile([65, 512], F32, tag=f'o{lc}', name=f'o65_{lc}') for lc in range(NLC)]

            def emit_stats(lc):
                for st in stat_steps(lc):
                    st()

            emit_stats(0)
            emit_brow(0)
            emit_stats(1)

            def emit_norm(lc):
                r1 = scr.tile([1, 512], F32, tag='r1', name=f'r1_{lc}')
                s2 = scr.tile([1, 512], F32, tag='s2', name=f's2_{lc}')
                nc.vector.tensor_copy(s2[:], o65[lc][64:65, :])
                nc.vector.reciprocal_approx_fast(r1[:], s2[:])
                r2 = scr.tile([64, 512], F32, tag='r2', name=f'r2_{lc}')
                nc.gpsimd.partition_broadcast(r2[:], r1[:])
                ofin = scr.tile([64, 512], F32, tag='ofin', name=f'of{lc}')
                nc.vector.tensor_tensor(ofin[:], o65[lc][0:64, :], r2[:], op=mybir.AluOpType.mult)
                nc.sync.dma_start(out_p[0:64, lc * 512:(lc + 1) * 512], ofin[:])

            for lc in range(NLC):
                # C(lc): per jt-pair: E^T x2 into one [128,1024] psum, one exp, two AV
                # MMs; interleave next l-chunk's stat matmuls 1:1 so the in-order PE
                # queue never stalls on a long run of stat work.
                sl = slice(lc * 512, (lc + 1) * 512)
                pts = []
                for jt in range(NJT):
                    e = wkp.tile([128, 512], F32, tag='wk', name=f'e{lc}_{jt}')
                    nc.tensor.matmul(e[:], K65[0:65, jt * 128:(jt + 1) * 128],
                                     Q65[0:65, sl], start=True, stop=True)
                    pt = ptp.tile([128, 512], BF16, tag='pt', name=f'pt{lc}_{jt}')
                    nc.scalar.activation(pt[:], e[:], mybir.ActivationFunctionType.Exp,
                                         bias=0.0, scale=SCALE)
                    pts.append(pt)
                    if jt > 0:
                        nc.tensor.matmul(o65[lc][:], vt65[:, (jt - 1) * 65:jt * 65],
                                         pts[jt - 1][:],
                                         start=(jt == 1), stop=False)
                nc.tensor.matmul(o65[lc][:], vt65[:, (NJT - 1) * 65:NJT * 65], pts[-1][:],
                                 start=False, stop=True)
                if lc + 1 < NLC:
                    emit_brow(lc + 1)
                if lc + 2 < NLC:
                    emit_stats(lc + 2)
                emit_norm(lc)

            # v output channels: straight DMA (f32r bits are valid f32)
            nc.sync.dma_start(out_p[64:128, :], v_sb[:].bitcast(F32))

    nc.finalize()
    return nc


_cache = {}


def _get_nc(gamma: float):
    key = float(gamma)
    if key not in _cache:
        _cache[key] = build_nc(key)
    return _cache[key]


def _in_maps(inputs):
    x = np.asarray(inputs['x'], np.float32)
    vwT = f32r_round(np.asarray(inputs['value_w'], np.float32).T)
    qwT = f32r_round(np.asarray(inputs['query_w'], np.float32).T / SCALE)
    kwT = f32r_round(np.asarray(inputs['key_w'], np.float32).T)
    ident = np.eye(128, dtype=np.float32)
    xs = f32r_round(x[..., 0])
    return [
        {'x': np.ascontiguousarray(xs[b]), 'vwT': vwT, 'qwT': qwT, 'kwT': kwT, 'ident': ident}
        for b in range(B)
    ]


def kernel(x, value_w, value_b, query_w, query_b, key_w, key_b, gamma):
    gamma_f = float(np.asarray(gamma).reshape(-1)[0])
    nc = _get_nc(gamma_f)
    maps = _in_maps(dict(x=x, value_w=value_w, query_w=query_w, key_w=key_w))
    res = run_bass_kernel_spmd(nc, maps, core_ids=list(range(B)), trace=False)
    out = np.stack([res.results[b]['out'] for b in range(B)], axis=0)
    return out[..., None].astype(np.float32)


def run_traced(inputs):
    gamma_f = float(np.asarray(inputs['gamma']).reshape(-1)[0])
    nc = _get_nc(gamma_f)
    maps = _in_maps(inputs)
    res = run_bass_kernel_spmd(nc, maps, core_ids=list(range(B)), trace=True)
    out = np.stack([res.results[b]['out'] for b in range(B)], axis=0)
    return out[..., None].astype(np.float32), res.exec_time_ns
